# revision 22
# baseline (speedup 1.0000x reference)
"""Class-balanced cross-entropy loss kernel for Trainium2 (8 NeuronCores).

Problem: output [4,8,64,128,128] f32 logits, labels [4,1,64,128,128] int
(values 0..7).  loss = mean over present classes of (per-class mean CE).

Strategy (data-parallel over the flattened voxel axis, 524288 voxels/core):
  per-voxel CE loss  l_i = logsumexp_c(x_ic) - x_i[lab_i]
  per-class sums     sums[c]  = S_lse[c] - S_g[c]
     S_lse[c] = sum_{i: lab=c} lse_i      (masked accumulate, DVE)
     S_g[c]   = sum_{i: lab=c} x_i[c]     (masked accumulate, DVE)
     counts[c]                            (masked accumulate, DVE)
  final scalar combined on host from tiny per-core partials.

Device layout per core: 4 superblocks of 8 slabs (slab = H*W = 16384 vox).
  x tiles  [128, 4096] bf16, two per superblock (class halves):
     xlo[chat*32+v1, shat*512+v2] = x[b, chat,   d, v1, v2]   chat in 0..3
     xhi[...]                     = x[b, chat+4, d, v1, v2]
  exp on ACT; s = sum over 8 classes via two PE matmuls (G32 stationary
  group-sum matrix, second matmul accumulates with start=False) -> PSUM.
  lse = log(s) on ACT -> per-core [128, 4096] bf16 buffer.
  Masked per-class accumulations via scalar_tensor_tensor / tensor_scalar
  with fused per-partition accum_out (bf16 operands -> 2x/4x DVE modes).
"""

import numpy as np
import ml_dtypes

import concourse.bass as bass
import concourse.bacc as bacc
import concourse.mybir as mybir
from concourse import bass_utils, tile

BF16 = mybir.dt.bfloat16
F32 = mybir.dt.float32
NPBF16 = ml_dtypes.bfloat16

N_CORES = 8
B, C, D, H, W = 4, 8, 64, 128, 128
N_SB = 4                                # superblocks per core
SB_COLS = 4096
VOX_PER_CORE = 32 * H * W               # 524288

_PROG_CACHE = {}


def _build_program():
    nc = bacc.Bacc("TRN2", target_bir_lowering=False, debug=False)

    x_in = nc.dram_tensor("x", [N_SB, 2, 128, SB_COLS], BF16, kind="ExternalInput")
    lr_in = nc.dram_tensor("labrep", [N_SB, 128, SB_COLS], BF16, kind="ExternalInput")
    ll_in = nc.dram_tensor("lablse", [128, SB_COLS], BF16, kind="ExternalInput")
    g32_in = nc.dram_tensor("g32", [128, 32], BF16, kind="ExternalInput")
    pm4_in = nc.dram_tensor("pm4", [128, 2], F32, kind="ExternalInput")
    hbias_in = nc.dram_tensor("hbias", [128, 7], F32, kind="ExternalInput")
    out_d = nc.dram_tensor("partials", [128, 39], F32, kind="ExternalOutput")

    with tile.TileContext(nc) as tc:
        with (
            tc.tile_pool(name="const", bufs=1) as cpool,
            tc.tile_pool(name="io", bufs=3) as iopool,
            tc.tile_pool(name="work", bufs=2) as wpool,
            tc.tile_pool(name="psum", bufs=8, space="PSUM") as ppool,
        ):
            eq = mybir.AluOpType.is_equal
            mul = mybir.AluOpType.mult

            # sb0's first half goes out before everything else so compute
            # can start as early as possible (single HW queue, program order)
            xlo0 = iopool.tile([128, SB_COLS], BF16, tag="xlo")
            lr0 = iopool.tile([128, SB_COLS], BF16, tag="lr")
            xhi0 = iopool.tile([128, SB_COLS], BF16, tag="xhi")
            h0 = slice(0, SB_COLS // 2)
            h1 = slice(SB_COLS // 2, SB_COLS)
            nc.sync.dma_start(xlo0[:, h0], x_in[0, 0][:, h0])
            nc.sync.dma_start(lr0[:, h0], lr_in[0][:, h0])

            g32 = cpool.tile([128, 32], BF16)
            nc.sync.dma_start(g32[:], g32_in[:])
            pm4 = cpool.tile([128, 2], F32)
            nc.sync.dma_start(pm4[:], pm4_in[:])
            hbias = cpool.tile([128, 7], F32)
            nc.sync.dma_start(hbias[:], hbias_in[:])

            nc.sync.dma_start(xhi0[:, h0], x_in[0, 1][:, h0])
            nc.sync.dma_start(xlo0[:, h1], x_in[0, 0][:, h1])
            nc.sync.dma_start(lr0[:, h1], lr_in[0][:, h1])
            nc.sync.dma_start(xhi0[:, h1], x_in[0, 1][:, h1])

            lab_lse = cpool.tile([128, SB_COLS], BF16)
            nc.sync.dma_start(lab_lse[:], ll_in[:])
            lse = cpool.tile([128, SB_COLS], BF16)
            sg_acc = cpool.tile([128, 10], F32)
            slse_acc = cpool.tile([128, 14], F32)
            cnt_acc = cpool.tile([128, 7], F32)
            glse_acc = cpool.tile([128, 8], F32)

            # tiny reads that absorb DMA semaphore waits so wait-slot-limited
            # ops need at most one wait
            dummy = cpool.tile([128, 4], F32)
            nc.vector.tensor_copy(dummy[:, 0:2], pm4[:])
            nc.vector.tensor_copy(dummy[:, 2:3], lab_lse[:, 0:1])
            nc.scalar.activation(
                dummy[:, 3:4], hbias[:, 0:1], mybir.ActivationFunctionType.Copy
            )

            pstiles = []
            for sb in range(N_SB):
                if sb == 0:
                    xlo, xhi, lr_sb = xlo0, xhi0, lr0
                    chunks = (h0, h1)
                else:
                    xlo = iopool.tile([128, SB_COLS], BF16, tag="xlo")
                    xhi = iopool.tile([128, SB_COLS], BF16, tag="xhi")
                    lr_sb = iopool.tile([128, SB_COLS], BF16, tag="lr")
                    nc.sync.dma_start(xlo[:], x_in[sb, 0])
                    nc.sync.dma_start(lr_sb[:], lr_in[sb])
                    nc.sync.dma_start(xhi[:], x_in[sb, 1])
                    chunks = (slice(0, SB_COLS),)
                nc.vector.tensor_copy(dummy[:, 3:4], lr_sb[:, 0:1])

                # S_g partials + exp, per chunk
                elo = wpool.tile([128, SB_COLS], BF16, tag="elo")
                ehi = wpool.tile([128, SB_COLS], BF16, tag="ehi")
                for ci, cs in enumerate(chunks):
                    for h, x_sb in ((0, xlo), (1, xhi)):
                        col = (2 * sb + h) if ci == 0 else (8 + h)
                        sc = wpool.tile([128, SB_COLS], BF16, tag="sc")
                        nc.vector.scalar_tensor_tensor(
                            sc[:, cs],
                            lr_sb[:, cs],
                            pm4[:, h : h + 1],
                            x_sb[:, cs],
                            eq,
                            mul,
                            accum_out=sg_acc[:, col : col + 1],
                        )
                    nc.scalar.activation(
                        elo[:, cs], xlo[:, cs], mybir.ActivationFunctionType.Exp
                    )
                    nc.scalar.activation(
                        ehi[:, cs], xhi[:, cs], mybir.ActivationFunctionType.Exp
                    )

                # class-group sums on PE into per-superblock psum tiles
                for g in range(2):
                    ps = ppool.tile([128, 512], F32, tag="ps")
                    for q in range(4):
                        sl = 512 * (4 * g + q)
                        nc.tensor.matmul(
                            ps[32 * q : 32 * (q + 1), :],
                            g32[:],
                            elo[:, sl : sl + 512],
                            start=True,
                            stop=False,
                            tile_position=(0, 32 * q),
                        )
                        nc.tensor.matmul(
                            ps[32 * q : 32 * (q + 1), :],
                            g32[:],
                            ehi[:, sl : sl + 512],
                            start=False,
                            stop=True,
                            tile_position=(0, 32 * q),
                        )
                    pstiles.append((2 * sb + g, ps))

                if sb % 2 == 0:
                    # count functionals in ACT's slack:
                    # m_j = sum_i sign(lab_i - j + 0.5) = N - 2*cum_count(j);
                    # sign is exact (+-1) and lives in every ACT table set.
                    for j in (sb, sb + 1):
                        sc3 = wpool.tile([128, SB_COLS], BF16, tag="sc3")
                        nc.scalar.activation(
                            sc3[:],
                            lab_lse[:],
                            mybir.ActivationFunctionType.Sign,
                            bias=hbias[:, j : j + 1],
                            accum_out=cnt_acc[:, j : j + 1],
                        )
                    continue

                # end of a pair: batched lns (one table load), then the
                # per-class masked lse sums over the pair's [128, 2048] slice
                pair = sb // 2
                for u, ps in pstiles:
                    nc.scalar.activation(
                        lse[:, 512 * u : 512 * (u + 1)],
                        ps[:],
                        mybir.ActivationFunctionType.Ln,
                        accum_out=glse_acc[:, u : u + 1],
                    )
                pstiles = []
                lsl = lse[:, 2048 * pair : 2048 * (pair + 1)]
                lll = lab_lse[:, 2048 * pair : 2048 * (pair + 1)]
                for c in range(7):
                    sc2 = wpool.tile([128, 2048], BF16, tag="sc2")
                    nc.vector.scalar_tensor_tensor(
                        sc2[:],
                        lll,
                        float(c),
                        lsl,
                        eq,
                        mul,
                        accum_out=slse_acc[:, 7 * pair + c : 7 * pair + c + 1],
                    )

            # remaining count functionals
            for j in (4, 5, 6):
                sc3 = wpool.tile([128, SB_COLS], BF16, tag="sc3")
                nc.scalar.activation(
                    sc3[:],
                    lab_lse[:],
                    mybir.ActivationFunctionType.Sign,
                    bias=hbias[:, j : j + 1],
                    accum_out=cnt_acc[:, j : j + 1],
                )

            nc.sync.dma_start(out_d[:, 0:10], sg_acc[:])
            nc.sync.dma_start(out_d[:, 10:24], slse_acc[:])
            nc.sync.dma_start(out_d[:, 24:31], cnt_acc[:])
            nc.sync.dma_start(out_d[:, 31:39], glse_acc[:])

    nc.compile()
    return nc


def _host_prep(output, labels):
    """Build per-core input maps (sharding + layout prep, no math)."""
    x = np.asarray(output)
    lab = np.asarray(labels).astype(np.int32)

    g32 = np.zeros((128, 32), dtype=NPBF16)
    for ch in range(4):
        for v1 in range(32):
            g32[ch * 32 + v1, v1] = 1.0
    pcls = np.arange(128, dtype=np.int32) // 32
    pm4 = np.stack([pcls, pcls + 4], axis=1).astype(np.float32)

    in_maps = []
    for k in range(N_CORES):
        b, d0 = k // 2, 32 * (k % 2)
        # [8c, 4sb, 8shat, 32v1, 512v2] -> [sb, chat, v1, shat, v2]
        xc = x[b, :, d0 : d0 + 32].reshape(8, 4, 8, 32, 512)
        xt = xc.transpose(1, 0, 3, 2, 4).astype(NPBF16)  # [sb, c, v1, shat, v2]
        x_prep = np.stack(
            [
                np.ascontiguousarray(xt[:, :4]).reshape(4, 128, 4096),
                np.ascontiguousarray(xt[:, 4:]).reshape(4, 128, 4096),
            ],
            axis=1,
        )

        lc = lab[b, 0, d0 : d0 + 32].reshape(4, 8, 32, 512).astype(NPBF16)
        # labrep[sb, chat*32+v1, shat*512+v2]
        lr = lc.transpose(0, 2, 1, 3).reshape(4, 1, 32, 4096)
        lr = np.ascontiguousarray(
            np.broadcast_to(lr, (4, 4, 32, 4096))
        ).reshape(4, 128, 4096)
        # lablse[(shat%4)*32+v1, (2*sb + shat//4)*512+v2]
        l2 = lc.reshape(4, 2, 4, 32, 512)  # [sb, sh, sl, v1, v2]
        ll = np.ascontiguousarray(l2.transpose(2, 3, 0, 1, 4)).reshape(128, 4096)

        in_maps.append(
            {
                "x": x_prep,
                "labrep": lr,
                "lablse": ll,
                "g32": g32,
                "pm4": pm4,
                "hbias": np.broadcast_to(
                    0.5 - np.arange(1, 8, dtype=np.float32), (128, 7)
                ).copy(),
            }
        )
    return in_maps


def _combine(results):
    """Host gather: reduce per-core [3,128,8] partials to the final scalar."""
    S_g = np.zeros(8, dtype=np.float64)
    S_lse = np.zeros(8, dtype=np.float64)
    cnt = np.zeros(8, dtype=np.float64)
    pclass = np.arange(128) // 32  # 0..3 per partition
    m = np.zeros(7, dtype=np.float64)
    glse = 0.0
    n_total = 0
    for r in results:
        p = np.asarray(r["partials"], dtype=np.float64)
        sg, slse, cn = p[:, 0:10], p[:, 10:24], p[:, 24:31]
        lo_cols, hi_cols = [0, 2, 4, 6, 8], [1, 3, 5, 7, 9]
        for ch in range(4):
            rows = pclass == ch
            S_g[ch] += sg[np.ix_(rows, lo_cols)].sum()
            S_g[ch + 4] += sg[np.ix_(rows, hi_cols)].sum()
        sl = slse.sum(axis=0).reshape(2, 7).sum(axis=0)
        S_lse[:7] += sl
        glse += p[:, 31:39].sum()
        m += cn.sum(axis=0)
        n_total += VOX_PER_CORE
    S_lse[7] = glse - S_lse[:7].sum()
    # histogram from sign-staircase functionals (exact +-1 entries)
    js = np.arange(1, 8, dtype=np.float64)
    A = np.vstack(
        [np.ones(8), np.sign(np.arange(8)[None, :] - js[:, None] + 0.5)]
    )
    cnt[:] = np.round(np.linalg.solve(A, np.concatenate([[n_total], m])))
    sums = S_lse - S_g
    present = cnt > 0
    class_means = sums / np.maximum(cnt, 1.0)
    n_valid = present.sum()
    loss = np.where(present, class_means, 0.0).sum() / n_valid
    return np.float32(loss)


def run(inputs_maps=None, trace=False, **inputs):
    if "nc" not in _PROG_CACHE:
        _PROG_CACHE["nc"] = _build_program()
    nc = _PROG_CACHE["nc"]
    in_maps = inputs_maps if inputs_maps is not None else _host_prep(**inputs)
    res = bass_utils.run_bass_kernel_spmd(
        nc, in_maps, list(range(N_CORES)), trace=trace
    )
    return res


def kernel(output, labels):
    res = run(output=output, labels=labels)
    return _combine(res.results)
